# revision 4
# baseline (speedup 1.0000x reference)
"""BiLSTM Trainium2 kernel (nn_BiLSTM_16707422781942).

Strategy: 8 cores = 4 forward-direction cores + 4 backward-direction cores,
each handling a batch slice of 16 (of B=64). The backward direction is fed
time-reversed input, making the device program direction-agnostic (pure SPMD).

Per core:
  Phase A (projection): xg[t] = W_ih^T x_t + (b_ih + b_hh), all t, as bf16
     matmuls with gate-dim on PSUM partitions; spilled to DRAM in
     [c=32][p=128][t=512][b=16] layout (c = 128-row chunk of the 4H gate dim).
  Phase B (recurrence): per step, gates.T [4H x 16] computed as 4 gate PSUM
     banks [128, 8, 16]; xg_t injected into PSUM via identity matmul, then
     64 accumulating bf16 matmuls per gate (8 m-chunks x 8 k-chunks) of
     W_hh^T against h_{t-1}; sigmoid/tanh on ScalarE, cell update on VectorE.

Shapes (hardcoded): T=512, B=64, E=H=1024, 4H=4096. B_local=16.
"""

import numpy as np
import ml_dtypes

import concourse.bass as bass
import concourse.mybir as mybir
import concourse.tile as tile
from concourse import bacc
from concourse.bass import ts, ds
from concourse.bass_utils import run_bass_kernel_spmd
from concourse.tile import TileContext
from concourse.masks import make_identity

F32 = mybir.dt.float32
BF16 = mybir.dt.bfloat16
AF = mybir.ActivationFunctionType

T, B, E, H = 512, 64, 1024, 1024
BL = 16          # batch per core
NC = 8           # cores
KC = E // 128    # 8 contraction chunks
GC = (4 * H) // 128  # 32 gate-dim chunks
HC = H // 128    # 8 hidden chunks
TB = 16          # time blocks in projection (each 32 steps)
TBS = T // TB    # 32 steps per block


def build_program(t_steps=T, repeat=1, fori=False):
    nc = bacc.Bacc("TRN2", target_bir_lowering=False, debug=False, num_devices=NC)

    xt_d = nc.dram_tensor("xt", [E, t_steps, BL], BF16, kind="ExternalInput").ap()
    wit_d = nc.dram_tensor("wit", [E, 4 * H], BF16, kind="ExternalInput").ap()
    wht_d = nc.dram_tensor("wht", [H, 4 * H], BF16, kind="ExternalInput").ap()
    bias_d = nc.dram_tensor("bias", [128, GC], F32, kind="ExternalInput").ap()
    h0_d = nc.dram_tensor("h0", [128, HC, BL], BF16, kind="ExternalInput").ap()
    c0_d = nc.dram_tensor("c0", [128, HC, BL], F32, kind="ExternalInput").ap()
    hs_d = nc.dram_tensor("hs", [t_steps, HC, 128, BL], F32, kind="ExternalOutput").ap()

    n_tb = t_steps // TBS
    assert t_steps % TBS == 0

    with TileContext(nc) as tc:
        with (
            tc.tile_pool(name="wpool", bufs=1) as wpool,
            tc.tile_pool(name="dram", bufs=1, space="DRAM") as dpool,
            tc.tile_pool(name="xin", bufs=3) as xin,
            tc.tile_pool(name="xgout", bufs=4) as xgout,
            tc.tile_pool(name="pproj", bufs=4, space="PSUM") as pproj,
            tc.tile_pool(name="state", bufs=2) as state,
            tc.tile_pool(name="vec", bufs=2) as vec,
            tc.tile_pool(name="xg", bufs=4) as xgpool,
            tc.tile_pool(name="pgate", bufs=1, space="PSUM") as pgate,
        ):
            # ---- static tiles ----
            wit_sb = wpool.tile([128, KC, 4 * H], BF16)
            nc.sync.dma_start(wit_sb[:], wit_d.rearrange("(ko p) m -> p ko m", p=128))
            wht_sb = wpool.tile([128, KC, 4 * H], BF16)
            nc.sync.dma_start(wht_sb[:], wht_d.rearrange("(ko p) m -> p ko m", p=128))
            bias_sb = wpool.tile([128, GC], F32)
            nc.sync.dma_start(bias_sb[:], bias_d[:])
            ident = wpool.tile([128, 128], BF16)
            make_identity(nc, ident[:])

            # xg scratch in DRAM: [c][p][t][b]
            xg_dram = dpool.tile([GC, 128, t_steps, BL], BF16)

            import contextlib

            def _rep_ctx():
                if fori:
                    return tc.For_i(0, repeat, 1)
                return contextlib.nullcontext(0)

            with _rep_ctx():
              for _rep in range(1 if fori else repeat):
                # ================= Phase A: input projection =================
                for tb in range(n_tb):
                    xsb = xin.tile([128, KC, TBS * BL], BF16, tag="xsb")
                    nc.sync.dma_start(
                        xsb[:],
                        xt_d[:, ts(tb, TBS), :].rearrange(
                            "(ko p) t b -> p ko (t b)", p=128
                        ),
                    )
                    for c in range(GC):
                        pt = pproj.tile([128, TBS * BL], F32, tag="pp")
                        for k in range(KC):
                            nc.tensor.matmul(
                                pt[:],
                                wit_sb[:, k, ts(c, 128)],
                                xsb[:, k, :],
                                start=(k == 0),
                                stop=(k == KC - 1),
                            )
                        xg_sb = xgout.tile([128, TBS * BL], BF16, tag="xgs")
                        nc.scalar.activation(
                            xg_sb[:], pt[:], AF.Identity, bias=bias_sb[:, c : c + 1]
                        )
                        nc.sync.dma_start(
                            xg_dram[c, :, ts(tb, TBS), :].rearrange("p t b -> p (t b)"),
                            xg_sb[:],
                        )

                # ================= Phase B: recurrence =================
                h_prev = state.tile([128, HC, BL], BF16, tag="h")
                nc.sync.dma_start(h_prev[:], h0_d[:])
                c_prev = state.tile([128, HC, BL], F32, tag="c")
                nc.sync.dma_start(c_prev[:], c0_d[:])

                for s in range(t_steps):
                    xg4 = xgpool.tile([128, GC, BL], BF16, tag="xg")
                    nc.sync.dma_start(
                        xg4[:], xg_dram[:, :, s, :].rearrange("c p b -> p c b")
                    )

                    # gate order: f, i, g, o  (o last -> shortest critical tail)
                    # gate index in the 4H dim: i=0, f=1, g=2, o=3
                    pg = {}
                    for gname, gi in (("f", 1), ("i", 0), ("g", 2), ("o", 3)):
                        P_g = pgate.tile([128, HC, BL], F32, tag=f"pg_{gname}")
                        pg[gname] = P_g
                        nc.tensor.matmul(
                            P_g[:],
                            ident[:],
                            xg4[:, ds(gi * HC, HC), :],
                            start=True,
                            stop=False,
                        )
                        for hc in range(HC):
                            m = gi * HC + hc
                            for k in range(KC):
                                nc.tensor.matmul(
                                    P_g[:, hc, :],
                                    wht_sb[:, k, ds(m * 128, 128)],
                                    h_prev[:, k, :],
                                    start=False,
                                    stop=(k == KC - 1),
                                )

                    Fg = vec.tile([128, HC, BL], F32, tag="F")
                    nc.scalar.activation(Fg[:], pg["f"][:], AF.Sigmoid)
                    Ig = vec.tile([128, HC, BL], F32, tag="I")
                    nc.scalar.activation(Ig[:], pg["i"][:], AF.Sigmoid)
                    Gg = vec.tile([128, HC, BL], F32, tag="G")
                    nc.scalar.activation(Gg[:], pg["g"][:], AF.Tanh)

                    t1 = vec.tile([128, HC, BL], F32, tag="t1")
                    nc.vector.tensor_tensor(t1[:], Fg[:], c_prev[:], mybir.AluOpType.mult)
                    t2 = vec.tile([128, HC, BL], F32, tag="t2")
                    nc.vector.tensor_tensor(t2[:], Ig[:], Gg[:], mybir.AluOpType.mult)
                    c_new = state.tile([128, HC, BL], F32, tag="c")
                    nc.vector.tensor_tensor(c_new[:], t1[:], t2[:], mybir.AluOpType.add)

                    Tc = vec.tile([128, HC, BL], F32, tag="Tc")
                    nc.scalar.activation(Tc[:], c_new[:], AF.Tanh)

                    Og = vec.tile([128, HC, BL], F32, tag="O")
                    nc.scalar.activation(Og[:], pg["o"][:], AF.Sigmoid)

                    h_f32 = vec.tile([128, HC, BL], F32, tag="hf")
                    nc.vector.tensor_tensor(h_f32[:], Og[:], Tc[:], mybir.AluOpType.mult)
                    h_new = state.tile([128, HC, BL], BF16, tag="h")
                    nc.vector.tensor_copy(h_new[:], h_f32[:])

                    nc.sync.dma_start(
                        hs_d[s].rearrange("hc p b -> p hc b"), h_f32[:]
                    )

                    h_prev, c_prev = h_new, c_new

    nc.compile()
    return nc


_cached = {}


def _get_program(t_steps=T, repeat=1):
    key = (t_steps, repeat)
    if key not in _cached:
        _cached[key] = build_program(t_steps, repeat)
    return _cached[key]


def _prep_core_inputs(inp, init_hidd, init_cell, W_ih, W_hh, b_ih, b_hh, bs, rev,
                      t_steps=T):
    """Host-side prep of one core's input map."""
    x = inp[::-1] if rev else inp
    x = x[:t_steps, bs, :]                       # [t, 16, E]
    xt = np.ascontiguousarray(x.transpose(2, 0, 1)).astype(ml_dtypes.bfloat16)
    wit = np.ascontiguousarray(W_ih.T).astype(ml_dtypes.bfloat16)   # [E, 4H]
    wht = np.ascontiguousarray(W_hh.T).astype(ml_dtypes.bfloat16)   # [H, 4H]
    bias = np.ascontiguousarray((b_ih + b_hh).reshape(GC, 128).T).astype(np.float32)
    h0 = np.ascontiguousarray(
        init_hidd[bs].reshape(BL, HC, 128).transpose(2, 1, 0)
    ).astype(ml_dtypes.bfloat16)                 # [128, hc, b]
    c0 = np.ascontiguousarray(
        init_cell[bs].reshape(BL, HC, 128).transpose(2, 1, 0)
    ).astype(np.float32)
    return {"xt": xt, "wit": wit, "wht": wht, "bias": bias, "h0": h0, "c0": c0}


def run_cores(inputs, t_steps=T, repeat=1):
    """Run the SPMD program; returns list of per-core HS arrays [t, hc, 128, b]."""
    nc = _get_program(t_steps, repeat)
    in_maps = []
    for d in range(2):          # 0 = fw, 1 = bw
        w = ("fw", "bw")[d]
        for j in range(4):
            bs = slice(16 * j, 16 * (j + 1))
            in_maps.append(
                _prep_core_inputs(
                    inputs["inp"], inputs["init_hidd"], inputs["init_cell"],
                    inputs[f"W_ih_{w}"], inputs[f"W_hh_{w}"],
                    inputs[f"b_ih_{w}"], inputs[f"b_hh_{w}"],
                    bs, rev=(d == 1), t_steps=t_steps,
                )
            )
    res = run_bass_kernel_spmd(nc, in_maps, list(range(NC)))
    return [res.results[i]["hs"] for i in range(NC)]


def kernel(inp, init_hidd, init_cell,
           W_ih_fw, W_hh_fw, b_ih_fw, b_hh_fw,
           W_ih_bw, W_hh_bw, b_ih_bw, b_hh_bw):
    inputs = dict(
        inp=np.asarray(inp), init_hidd=np.asarray(init_hidd),
        init_cell=np.asarray(init_cell),
        W_ih_fw=np.asarray(W_ih_fw), W_hh_fw=np.asarray(W_hh_fw),
        b_ih_fw=np.asarray(b_ih_fw), b_hh_fw=np.asarray(b_hh_fw),
        W_ih_bw=np.asarray(W_ih_bw), W_hh_bw=np.asarray(W_hh_bw),
        b_ih_bw=np.asarray(b_ih_bw), b_hh_bw=np.asarray(b_hh_bw),
    )
    hs_list = run_cores(inputs)

    hid = np.empty((B, T + 1, 2 * H), dtype=np.float32)
    # forward: cores 0-3; HS[s] = fw state after consuming inp[0..s] -> hid[:, s, :H]
    for j in range(4):
        hs = hs_list[j]  # [t, hc, p, b]
        hb = hs.transpose(3, 0, 1, 2).reshape(BL, T, H)  # [b, t, h]
        hid[16 * j : 16 * (j + 1), :T, :H] = hb
    hid[:, T, :H] = inputs["init_hidd"]
    # backward: cores 4-7; HS[s] = bw state after consuming inp[T-1-s..T-1]
    # -> hid index j = T - s for s in 0..T-1 (i.e. hid[:, 1:T+1] reversed)
    for j in range(4):
        hs = hs_list[4 + j]
        hb = hs.transpose(3, 0, 1, 2).reshape(BL, T, H)  # [b, s, h]
        hid[16 * j : 16 * (j + 1), 1 : T + 1, H:] = hb[:, ::-1, :]
    hid[:, 0, H:] = inputs["init_hidd"]

    last_hidd = np.empty((B, 2 * H), dtype=np.float32)
    for j in range(4):
        last_hidd[16 * j : 16 * (j + 1), :H] = (
            hs_list[j][T - 1].transpose(2, 0, 1).reshape(BL, H)
        )
        last_hidd[16 * j : 16 * (j + 1), H:] = (
            hs_list[4 + j][T - 1].transpose(2, 0, 1).reshape(BL, H)
        )
    last_cell = last_hidd.copy()
    return hid, last_hidd, last_cell


# revision 13
# speedup vs baseline: 1.8160x; 1.8160x over previous
"""BiLSTM Trainium2 kernel (nn_BiLSTM_16707422781942).

Strategy: 8 cores = 4 forward-direction cores + 4 backward-direction cores,
each handling a batch slice of 16 (of B=64). The backward direction is fed
time-reversed input, making the device program direction-agnostic (pure SPMD).

Per core:
  Phase A (projection): xg[t] = W_ih^T x_t + (b_ih + b_hh), all t, as bf16
     matmuls with gate-dim on PSUM partitions; spilled to DRAM in
     [c=32][p=128][t=512][b=16] layout (c = 128-row chunk of the 4H gate dim).
  Phase B (recurrence): per step, gates.T [4H x 16] computed as 4 gate PSUM
     banks [128, 8, 16]; xg_t injected into PSUM via identity matmul, then
     64 accumulating bf16 matmuls per gate (8 m-chunks x 8 k-chunks) of
     W_hh^T against h_{t-1}; sigmoid/tanh on ScalarE, cell update on VectorE.

Shapes (hardcoded): T=512, B=64, E=H=1024, 4H=4096. B_local=16.
"""

import numpy as np
import ml_dtypes

import concourse.bass as bass
import concourse.mybir as mybir
import concourse.tile as tile
from concourse import bacc
from concourse.bass import ts, ds
from concourse.bass_utils import run_bass_kernel_spmd
from concourse.tile import TileContext
from concourse.masks import make_identity

F32 = mybir.dt.float32
BF16 = mybir.dt.bfloat16
AF = mybir.ActivationFunctionType

T, B, E, H = 512, 64, 1024, 1024
BL = 16          # batch per core
NC = 8           # cores
KC = E // 128    # 8 contraction chunks
GC = (4 * H) // 128  # 32 gate-dim chunks
HC = H // 128    # 8 hidden chunks
TB = 16          # time blocks in projection (each 32 steps)
TBS = T // TB    # 32 steps per block


def build_program(t_steps=T, repeat=1, fori=False, parts="all"):
    nc = bacc.Bacc("TRN2", target_bir_lowering=False, debug=False, num_devices=NC)

    xt_d = nc.dram_tensor("xt", [E, t_steps, BL], BF16, kind="ExternalInput").ap()
    wit_d = nc.dram_tensor("wit", [E, 4 * H], BF16, kind="ExternalInput").ap()
    wht_d = nc.dram_tensor("wht", [H, 4 * H], BF16, kind="ExternalInput").ap()
    bias_d = nc.dram_tensor("bias", [128, GC], F32, kind="ExternalInput").ap()
    h0_d = nc.dram_tensor("h0", [128, HC, BL], BF16, kind="ExternalInput").ap()
    c0_d = nc.dram_tensor("c0", [128, HC, BL], F32, kind="ExternalInput").ap()
    hs_d = nc.dram_tensor("hs", [t_steps, HC, 128, BL], F32, kind="ExternalOutput").ap()

    n_tb = t_steps // TBS
    assert t_steps % TBS == 0

    with TileContext(nc) as tc:
        with (
            tc.tile_pool(name="wpool", bufs=1) as wpool,
            tc.tile_pool(name="dram", bufs=1, space="DRAM") as dpool,
            tc.tile_pool(name="xin", bufs=3) as xin,
            tc.tile_pool(name="xgout", bufs=4) as xgout,
            tc.tile_pool(name="pproj", bufs=4, space="PSUM") as pproj,
            tc.tile_pool(name="state", bufs=2) as state,
            tc.tile_pool(name="vec", bufs=2) as vec,
            tc.tile_pool(name="xg", bufs=4) as xgpool,
            tc.tile_pool(name="pgate", bufs=1, space="PSUM") as pgate,
        ):
            # ---- static tiles ----
            wit_sb = wpool.tile([128, KC, 4 * H], BF16)
            nc.sync.dma_start(wit_sb[:], wit_d.rearrange("(ko p) m -> p ko m", p=128))
            wht_sb = wpool.tile([128, KC, 4 * H], BF16)
            nc.sync.dma_start(wht_sb[:], wht_d.rearrange("(ko p) m -> p ko m", p=128))
            bias_sb = wpool.tile([128, GC], F32)
            nc.sync.dma_start(bias_sb[:], bias_d[:])
            ident = wpool.tile([128, 128], BF16)
            make_identity(nc, ident[:])
            xg_fixed = None
            if parts in ("rec_noxg", "rec_nodma", "mm_pure", "vec_only"):
                xg_fixed = wpool.tile([128, GC, BL], BF16)
                nc.gpsimd.memset(xg_fixed[:], 0.125)
            pg_static = None
            if parts == "vec_only":
                pg_static = {}
                for gname, gi in (("f", 1), ("i", 0), ("g", 2), ("o", 3)):
                    P_g = wpool.tile([128, HC, BL], F32, space="PSUM", name=f"pgs_{gname}")
                    nc.tensor.matmul(P_g[:], ident[:], xg_fixed[:, ds(gi * HC, HC), :], start=True, stop=True)
                    pg_static[gname] = P_g

            # xg scratch in DRAM: [c][p][t][b]
            xg_dram = dpool.tile([GC, 128, t_steps, BL], BF16)

            import contextlib

            def _rep_ctx():
                if fori:
                    return tc.For_i(0, repeat, 1)
                return contextlib.nullcontext(0)

            with _rep_ctx():
              for _rep in range(1 if fori else repeat):
                # ================= Phase A: input projection =================
                for tb in range(n_tb if parts in ("all", "proj") else 0):
                    xsb = xin.tile([128, KC, TBS * BL], BF16, tag="xsb")
                    nc.sync.dma_start(
                        xsb[:],
                        xt_d[:, ts(tb, TBS), :].rearrange(
                            "(ko p) t b -> p ko (t b)", p=128
                        ),
                    )
                    for c in range(GC):
                        pt = pproj.tile([128, TBS * BL], F32, tag="pp")
                        for k in range(KC):
                            nc.tensor.matmul(
                                pt[:],
                                wit_sb[:, k, ts(c, 128)],
                                xsb[:, k, :],
                                start=(k == 0),
                                stop=(k == KC - 1),
                            )
                        xg_sb = xgout.tile([128, TBS * BL], BF16, tag="xgs")
                        nc.scalar.activation(
                            xg_sb[:], pt[:], AF.Identity, bias=bias_sb[:, c : c + 1]
                        )
                        nc.sync.dma_start(
                            xg_dram[c, :, ts(tb, TBS), :].rearrange("p t b -> p (t b)"),
                            xg_sb[:],
                        )

                # ================= Phase B: recurrence =================
                h_prev = state.tile([128, HC, BL], BF16, tag="h")
                nc.sync.dma_start(h_prev[:], h0_d[:])
                c_prev = state.tile([128, HC, BL], F32, tag="c")
                nc.sync.dma_start(c_prev[:], c0_d[:])

                for s in range(t_steps if parts != "proj" else 0):
                    if parts == "xg_dma_only":
                        xg4 = xgpool.tile([128, GC, BL], BF16, tag="xg")
                        nc.sync.dma_start(
                            xg4[:], xg_dram[:, :, s, :].rearrange("c p b -> p c b")
                        )
                        continue
                    if parts in ("rec_noxg", "rec_nodma", "mm_pure", "vec_only"):
                        xg4 = xg_fixed
                    else:
                        xg4 = xgpool.tile([128, GC, BL], BF16, tag="xg")
                        nc.sync.dma_start(
                            xg4[:], xg_dram[:, :, s, :].rearrange("c p b -> p c b")
                        )

                    if parts in ("mm_only", "mm_pure"):
                        for gname, gi in (("f", 1), ("i", 0), ("g", 2), ("o", 3)):
                            P_g = pgate.tile([128, HC, BL], F32, tag=f"pg_{gname}")
                            nc.tensor.matmul(P_g[:], ident[:], xg4[:, ds(gi * HC, HC), :], start=True, stop=False)
                            for hc in range(HC):
                                m = gi * HC + hc
                                for k in range(KC):
                                    nc.tensor.matmul(
                                        P_g[:, hc, :],
                                        wht_sb[:, k, ds(m * 128, 128)],
                                        h_prev[:, k, :],
                                        start=False, stop=(k == KC - 1),
                                    )
                        continue

                    # gate order: f, i, g, o  (o last -> shortest critical tail)
                    # gate index in the 4H dim: i=0, f=1, g=2, o=3
                    pg = {}
                    if parts == "vec_only":
                        pg = pg_static
                    GORDER = (() if parts == "vec_only" else (("f", 1), ("i", 0), ("g", 2), ("o", 3)))
                    for gname, gi in GORDER:
                        P_g = pgate.tile([128, HC, BL], F32, tag=f"pg_{gname}")
                        pg[gname] = P_g
                        nc.tensor.matmul(
                            P_g[:],
                            ident[:],
                            xg4[:, ds(gi * HC, HC), :],
                            start=True,
                            stop=False,
                        )
                    for gname, gi in GORDER:
                        P_g = pg[gname]
                        for hc in range(HC):
                            m = gi * HC + hc
                            for k in range(KC):
                                nc.tensor.matmul(
                                    P_g[:, hc, :],
                                    wht_sb[:, k, ds(m * 128, 128)],
                                    h_prev[:, k, :],
                                    start=False,
                                    stop=(k == KC - 1),
                                )

                    Fg = vec.tile([128, HC, BL], F32, tag="F")
                    nc.scalar.activation(Fg[:], pg["f"][:], AF.Sigmoid)
                    Ig = vec.tile([128, HC, BL], F32, tag="I")
                    nc.scalar.activation(Ig[:], pg["i"][:], AF.Sigmoid)
                    Gg = vec.tile([128, HC, BL], F32, tag="G")
                    nc.scalar.activation(Gg[:], pg["g"][:], AF.Tanh)

                    t1 = vec.tile([128, HC, BL], F32, tag="t1")
                    nc.vector.tensor_tensor(t1[:], Fg[:], c_prev[:], mybir.AluOpType.mult)
                    t2 = vec.tile([128, HC, BL], F32, tag="t2")
                    nc.vector.tensor_tensor(t2[:], Ig[:], Gg[:], mybir.AluOpType.mult)
                    c_new = state.tile([128, HC, BL], F32, tag="c")
                    nc.vector.tensor_tensor(c_new[:], t1[:], t2[:], mybir.AluOpType.add)

                    Tc = vec.tile([128, HC, BL], F32, tag="Tc")
                    nc.scalar.activation(Tc[:], c_new[:], AF.Tanh)

                    Og = vec.tile([128, HC, BL], F32, tag="O")
                    nc.scalar.activation(Og[:], pg["o"][:], AF.Sigmoid)

                    h_new = state.tile([128, HC, BL], BF16, tag="h")
                    nc.vector.tensor_tensor(h_new[:], Og[:], Tc[:], mybir.AluOpType.mult)
                    h_f32 = vec.tile([128, HC, BL], F32, tag="hf")
                    nc.vector.tensor_tensor(h_f32[:], Og[:], Tc[:], mybir.AluOpType.mult)

                    if parts != "rec_nodma":
                        nc.sync.dma_start(
                            hs_d[s].rearrange("hc p b -> p hc b"), h_f32[:]
                        )

                    h_prev, c_prev = h_new, c_new

    nc.compile()
    return nc


_cached = {}


def _get_program(t_steps=T, repeat=1):
    key = (t_steps, repeat)
    if key not in _cached:
        _cached[key] = build_program(t_steps, repeat)
    return _cached[key]


def _prep_core_inputs(inp, init_hidd, init_cell, W_ih, W_hh, b_ih, b_hh, bs, rev,
                      t_steps=T):
    """Host-side prep of one core's input map."""
    x = inp[::-1] if rev else inp
    x = x[:t_steps, bs, :]                       # [t, 16, E]
    xt = np.ascontiguousarray(x.transpose(2, 0, 1)).astype(ml_dtypes.bfloat16)
    wit = np.ascontiguousarray(W_ih.T).astype(ml_dtypes.bfloat16)   # [E, 4H]
    wht = np.ascontiguousarray(W_hh.T).astype(ml_dtypes.bfloat16)   # [H, 4H]
    bias = np.ascontiguousarray((b_ih + b_hh).reshape(GC, 128).T).astype(np.float32)
    h0 = np.ascontiguousarray(
        init_hidd[bs].reshape(BL, HC, 128).transpose(2, 1, 0)
    ).astype(ml_dtypes.bfloat16)                 # [128, hc, b]
    c0 = np.ascontiguousarray(
        init_cell[bs].reshape(BL, HC, 128).transpose(2, 1, 0)
    ).astype(np.float32)
    return {"xt": xt, "wit": wit, "wht": wht, "bias": bias, "h0": h0, "c0": c0}


def run_cores(inputs, t_steps=T, repeat=1):
    """Run the SPMD program; returns list of per-core HS arrays [t, hc, 128, b]."""
    nc = _get_program(t_steps, repeat)
    in_maps = []
    for d in range(2):          # 0 = fw, 1 = bw
        w = ("fw", "bw")[d]
        for j in range(4):
            bs = slice(16 * j, 16 * (j + 1))
            in_maps.append(
                _prep_core_inputs(
                    inputs["inp"], inputs["init_hidd"], inputs["init_cell"],
                    inputs[f"W_ih_{w}"], inputs[f"W_hh_{w}"],
                    inputs[f"b_ih_{w}"], inputs[f"b_hh_{w}"],
                    bs, rev=(d == 1), t_steps=t_steps,
                )
            )
    res = run_bass_kernel_spmd(nc, in_maps, list(range(NC)))
    return [res.results[i]["hs"] for i in range(NC)]


def kernel(inp, init_hidd, init_cell,
           W_ih_fw, W_hh_fw, b_ih_fw, b_hh_fw,
           W_ih_bw, W_hh_bw, b_ih_bw, b_hh_bw):
    inputs = dict(
        inp=np.asarray(inp), init_hidd=np.asarray(init_hidd),
        init_cell=np.asarray(init_cell),
        W_ih_fw=np.asarray(W_ih_fw), W_hh_fw=np.asarray(W_hh_fw),
        b_ih_fw=np.asarray(b_ih_fw), b_hh_fw=np.asarray(b_hh_fw),
        W_ih_bw=np.asarray(W_ih_bw), W_hh_bw=np.asarray(W_hh_bw),
        b_ih_bw=np.asarray(b_ih_bw), b_hh_bw=np.asarray(b_hh_bw),
    )
    hs_list = run_cores(inputs)

    hid = np.empty((B, T + 1, 2 * H), dtype=np.float32)
    # forward: cores 0-3; HS[s] = fw state after consuming inp[0..s] -> hid[:, s, :H]
    for j in range(4):
        hs = hs_list[j]  # [t, hc, p, b]
        hb = hs.transpose(3, 0, 1, 2).reshape(BL, T, H)  # [b, t, h]
        hid[16 * j : 16 * (j + 1), :T, :H] = hb
    hid[:, T, :H] = inputs["init_hidd"]
    # backward: cores 4-7; HS[s] = bw state after consuming inp[T-1-s..T-1]
    # -> hid index j = T - s for s in 0..T-1 (i.e. hid[:, 1:T+1] reversed)
    for j in range(4):
        hs = hs_list[4 + j]
        hb = hs.transpose(3, 0, 1, 2).reshape(BL, T, H)  # [b, s, h]
        hid[16 * j : 16 * (j + 1), 1 : T + 1, H:] = hb[:, ::-1, :]
    hid[:, 0, H:] = inputs["init_hidd"]

    last_hidd = np.empty((B, 2 * H), dtype=np.float32)
    for j in range(4):
        last_hidd[16 * j : 16 * (j + 1), :H] = (
            hs_list[j][T - 1].transpose(2, 0, 1).reshape(BL, H)
        )
        last_hidd[16 * j : 16 * (j + 1), H:] = (
            hs_list[4 + j][T - 1].transpose(2, 0, 1).reshape(BL, H)
        )
    last_cell = last_hidd.copy()
    return hid, last_hidd, last_cell
